# revision 8
# baseline (speedup 1.0000x reference)
"""GatedLinearAttention kernel for 8 Trainium2 NeuronCores.

Sharding (per the tensor-parallel hint):
  - 16 q-heads split 2 per core; each core also owns the single kv-head
    (of 4) that its 2 q-heads read (GQA group size 4 => cores {2d,2d+1}
    share kv head d//2).
  - qkv / gate projections computed per-core on the owned column slices
    (gate low-rank first matmul replicated, second matmul column-sharded).
  - Per-head recurrent scan runs chunk-parallel (chunk=128) locally.
  - o_proj is RowParallel: each core multiplies its 256 output channels
    against its 256-row slice of Wo, then psum all-reduce.
"""

import numpy as np
import jax
import jax.numpy as jnp
from functools import partial

T, H = 8192, 2048
NH, NKV, D = 16, 4, 128
R = 16
NORM = 16.0
EPS = 1e-6
C = 128                      # scan chunk length
NC = T // C
NDEV = 8
HPD = NH // NDEV             # q heads per device (2)

_TRIL = np.tril(np.ones((C, C), np.float32))


def _chunk_scan(q, k, v, g):
    """q:[T,HPD,D] (scaled, relu'd), k,v,g:[T,D] shared across local heads.
    Returns o:[T,HPD,D]."""
    b = jnp.einsum('ts,nsd->ntd', _TRIL, g.reshape(NC, C, D))  # inclusive cumsum per chunk
    eb = jnp.exp(b)
    kt = k.reshape(NC, C, D) * jnp.exp(-b)               # k * exp(-b)
    bC = b[:, -1]                                        # [NC, D]
    kd = k.reshape(NC, C, D) * jnp.exp(bC[:, None, :] - b)   # decay-to-chunk-end
    v_c = v.reshape(NC, C, D)
    q_c = q.reshape(NC, C, HPD, D)

    def step2(S, x):
        qc, ktc, kdc, vc, ebc, ebC = x
        qt = qc * ebc[:, None, :]
        A = jnp.einsum('thd,sd->hts', qt, ktc)
        A = A * _TRIL[None]
        o = jnp.einsum('hts,sd->thd', A, vc) + jnp.einsum('thd,hde->the', qt, S)
        S_new = jnp.exp(ebC)[:, None] * S + (kdc.T @ vc)[None]
        return S_new, o

    S0 = jnp.zeros((HPD, D, D), q.dtype)
    _, o = jax.lax.scan(step2, S0, (q_c, kt, kd, v_c, eb, bC))
    return o.reshape(T, HPD, D)


@partial(jax.pmap, axis_name='x')
def _core_fn(hidden, Wq, bq, Wk, bk, Wv, bv, gk, gnw, Wo_s):
    q = jnp.maximum(hidden @ Wq + bq, 0.0) * (D ** -0.5)     # [T, HPD*D]
    k = jnp.maximum(hidden @ Wk + bk, 0.0)                   # [T, D]
    v = hidden @ Wv + bv                                     # [T, D]

    o = _chunk_scan(q.reshape(T, HPD, D), k, v, gk)          # [T, HPD, D]
    o = o / jnp.sqrt(jnp.mean(o * o, axis=-1, keepdims=True) + EPS) * gnw
    out = o.reshape(T, HPD * D) @ Wo_s                       # [T, H]
    return jax.lax.psum(out, 'x')


def kernel(**inputs):
    hs = np.asarray(inputs['hidden_states'], np.float32)
    Wqkv = np.asarray(inputs['Wqkv'], np.float32)
    bqkv = np.asarray(inputs['bqkv'], np.float32)
    gw0 = np.asarray(inputs['gk_w0'], np.float32)
    gw1 = np.asarray(inputs['gk_w1'], np.float32)
    gb1 = np.asarray(inputs['gk_b1'], np.float32)
    gnw = np.asarray(inputs['gnorm_w'], np.float32)
    Wo = np.asarray(inputs['Wo'], np.float32)

    # per-device weight shards
    Wq_all = Wqkv[:, :NH * D].reshape(H, NDEV, HPD * D)
    bq_all = bqkv[:NH * D].reshape(NDEV, HPD * D)
    Wk_full = Wqkv[:, NH * D:(NH + NKV) * D]
    bk_full = bqkv[NH * D:(NH + NKV) * D]
    Wv_full = Wqkv[:, (NH + NKV) * D:]
    bv_full = bqkv[(NH + NKV) * D:]

    kv_of = [d // (NDEV // NKV) for d in range(NDEV)]        # [0,0,1,1,2,2,3,3]
    Wq_s = np.ascontiguousarray(Wq_all.transpose(1, 0, 2))                   # [8,H,256]
    bq_s = np.ascontiguousarray(bq_all)
    Wk_s = np.stack([Wk_full[:, g * D:(g + 1) * D] for g in kv_of])          # [8,H,128]
    bk_s = np.stack([bk_full[g * D:(g + 1) * D] for g in kv_of])
    Wv_s = np.stack([Wv_full[:, g * D:(g + 1) * D] for g in kv_of])
    bv_s = np.stack([bv_full[g * D:(g + 1) * D] for g in kv_of])
    Wo_s = Wo.reshape(NDEV, HPD * D, H)
    gnw_s = np.broadcast_to(gnw, (NDEV,) + gnw.shape)
    hs_s = np.broadcast_to(hs, (NDEV,) + hs.shape)

    # gate path on host: tiny low-rank matmul + log-sigmoid (neuronx-cc's
    # lower_act ICEs on log in this graph); gl is O(0.3) so plain form is stable
    gl = (hs @ gw0) @ gw1 + gb1                              # [T, NKV*D]
    gk_full = (-np.log1p(np.exp(-gl)) / NORM).astype(np.float32)
    gk_s = np.stack([gk_full[:, g * D:(g + 1) * D] for g in kv_of])  # [8,T,D]

    out = _core_fn(hs_s, Wq_s, bq_s, Wk_s, bk_s, Wv_s, bv_s,
                   gk_s, gnw_s, Wo_s)
    return np.asarray(out[0])


if __name__ == '__main__':
    import time
    rng = np.random.default_rng(0)
    ins = {
        'hidden_states': rng.standard_normal((T, H), np.float32),
        'Wqkv': rng.standard_normal((H, (NH + 2 * NKV) * D), np.float32) * 0.02,
        'bqkv': rng.standard_normal(((NH + 2 * NKV) * D,), np.float32) * 0.02,
        'gk_w0': rng.standard_normal((H, R), np.float32) * 0.02,
        'gk_w1': rng.standard_normal((R, NKV * D), np.float32) * 0.02,
        'gk_b1': rng.standard_normal((NKV * D,), np.float32) * 0.02,
        'gnorm_w': np.ones((D,), np.float32),
        'Wo': rng.standard_normal((NH * D, H), np.float32) * 0.02,
    }
    t0 = time.time(); out = kernel(**ins); t1 = time.time()
    print('out', out.shape, out.dtype, 'wall', t1 - t0)
    t0 = time.time(); out = kernel(**ins); t1 = time.time()
    print('second call wall', t1 - t0)
